# revision 36
# baseline (speedup 1.0000x reference)
"""Trainium2 Bass kernel for CenterWoParamMultiCosineLossV2.

Math (per sample b with label l):
    d_k   = 1 + <x_b, centers[l, k]>          k = 0..7
    value = (sum_k d_k^2) / (sum_k d_k)
    loss  = mean_b value
With u = sum_k <x_b, c_k> = <x_b, csum_l> and q = sum_k <x_b, c_k>^2:
    den = 8 + u,  num = 8 + 2u + q,  value = num / den

Only u needs fp32-grade precision (near-singular denominators: the
min |den| sample amplifies u error ~12000x into the loss); q tolerates
~0.1% error.  fp32r/bf16 matmuls round the moving x operand to ~11
mantissa bits (measured: u error 3.3e-2), so the main matmul must be
fp32 (LOW_HIGH, measured u error 4e-5).

Strategy (loss is a mean -> permutation invariant):
  * Host sorts samples by label; each of 8 cores takes 1024 consecutive
    sorted samples (~13 classes).  Per core, a class table [512, 128]
    holds, for each local class slot j: its 8 centers at columns
    8j..8j+7 and its center-sum at column M8+j  (M8 = 8*nslot).
  * One fp32 PE matmul chain per 512-sample half: S = table^T @ x^T in
    PSUM [128, 512]; rows 0..M8 are per-center scores s, rows M8..M9
    are u candidates per slot.
  * A tiny bf16 matmul expands the per-sample one-hot e [nslot, 512]
    through a fixed pattern G into a row mask [128, 512] (1 where the
    row belongs to the sample's slot).
  * tmp = S * mask (DVE);  tsq = tmp^2 in bf16 (= s^2*mask on center
    rows; bf16 is enough for q, ~3 abs error vs ~8 budget).
  * Two reduce matmuls accumulate into one PSUM bank per half: fp32
    over tmp (stationary 2/1 on csum rows -> 2u, u) + bf16 over tsq
    (stationary 1 on center rows -> q), yielding rows (q+2u, u); one
    8 KiB DMA returns the two rows per half.
  * Host folds the +8 and the division into its gather: loss =
    mean((q+2u+8)/(u+8)).
  * Post-passes work around walrus quirks and trim overhead: sem-range
    clears rewritten, tail barrier removed (engines halt after their
    own drains; SP's final waits cover the output DMA), TileContext
    sem-clears relocated to idle engines in `main`, and the first
    ct/xt00 DMA issues hoisted before the start barrier so HBM
    transfers overlap the reg-init preamble.
"""

import numpy as np
from contextlib import ExitStack

import concourse.bass as bass
import concourse.tile as tile
import concourse.mybir as mybir
from concourse import bass_utils

# ---------------------------------------------------------------------------
# Workaround: this walrus build accepts only ONE sem-wait per instruction
# ("Too many sync wait commands"), but Tile freely attaches several waits at
# join points.  Post-pass: for any instruction with k>1 waits, hoist k-1 of
# them onto same-engine nops inserted immediately before it.  Tile's per-
# engine stream is a projection of one topological order, so a producer's
# trigger always precedes a consumer's wait and engine-level blocking cannot
# deadlock; sequential waits on monotonic sems == simultaneous waits.
# ---------------------------------------------------------------------------
_SPLIT_ID = [0]


def _split_multi_waits(nc):
    for f in nc.m.functions:
        for blk in f.blocks:
            insts = blk.instructions
            for idx in range(len(insts) - 1, -1, -1):
                inst = insts[idx]
                si = inst.sync_info
                waits = list(si.on_wait or []) if si is not None else []
                if len(waits) <= 1:
                    continue
                # For DMA instructions, keep a COMPUTE dependency on the
                # instruction (it rides the queue descriptor, so the DMA
                # pipeline pre-runs while parked on the sem) and hoist the
                # early-firing queue-guard sems onto the engine nop.
                if type(inst).__name__ == "InstDMACopy":
                    comp = [
                        w
                        for w in waits
                        if not str(w.ant_name or "").startswith("DMA")
                    ]
                    if comp:
                        keep = comp[-1]
                        waits = [w for w in waits if w is not keep] + [keep]
                inst.sync_info = mybir.SyncInfo(
                    on_wait=[waits[-1]], on_update=list(si.on_update or [])
                )
                for w in reversed(waits[:-1]):
                    _SPLIT_ID[0] += 1
                    nop = mybir.InstNoOp(
                        name=f"I-waitsplit-{_SPLIT_ID[0]}", ins=[], outs=[]
                    )
                    nop.engine = inst.engine
                    nop.sync_info = mybir.SyncInfo(on_wait=[w], on_update=[])
                    insts.insert(idx, nop)


def _rewrite_range_clears(nc):
    """This walrus build rejects the EVENT_SEMAPHORE_RANGE_CLEAR raw-ISA
    encoding ("ISA wrong length"); replace each with per-sem
    InstEventSemaphore sem-wr-imm 0 writes on the same engine."""
    for f in nc.m.functions:
        for blk in f.blocks:
            insts = blk.instructions
            for idx in range(len(insts) - 1, -1, -1):
                inst = insts[idx]
                if type(inst).__name__ != "InstISA":
                    continue
                s = str(inst)
                if "EVENT_SEMAPHORE_RANGE_CLEAR" not in s:
                    continue
                import re

                first = int(re.search(r"range_first=(\d+)", s).group(1))
                last = int(re.search(r"range_last=(\d+)", s).group(1))
                si = inst.sync_info
                waits = list(si.on_wait or []) if si is not None else []
                upds = list(si.on_update or []) if si is not None else []
                repl = []
                for j, sem in enumerate(range(first, last + 1)):
                    _SPLIT_ID[0] += 1
                    ev = mybir.InstEventSemaphore(
                        name=f"I-semclr-{_SPLIT_ID[0]}", ins=[], outs=[]
                    )
                    ev.engine = inst.engine
                    ev.sync_info = mybir.SyncInfo(
                        on_wait=waits if j == 0 else [],
                        on_update=[
                            mybir.SyncUpdate(
                                sync_type="semaphore",
                                id=sem,
                                update_mode="sem-wr-imm",
                                update_value=0,
                            )
                        ]
                        + (upds if j == (last - first) else []),
                    )
                    repl.append(ev)
                insts[idx : idx + 1] = repl




def _hoist_first_dmas(nc):
    """Move the first HW-queue DMA of each of Activation/SP (ct, xt00) plus
    their sem clears to the top of `main`: they execute right after the
    walrus preamble, so the transfers overlap the reg-init + start barrier.
    Pool/SWDGE DMAs stay put (a hoisted software DMA stalls the barrier)."""
    f = nc.m.functions[0]
    blocks = {b.name: b for b in f.blocks}
    main = blocks["main"]
    tb = [b for n, b in blocks.items() if n.endswith("__build")][0]

    hoist = []
    seen = {}
    lim = {mybir.EngineType.Activation: 2}
    for inst in list(tb.instructions):
        if type(inst).__name__ != "InstDMACopy":
            continue
        eng = inst.engine
        if seen.get(eng, 0) >= lim.get(eng, 1) or eng == mybir.EngineType.Pool:
            continue
        si = inst.sync_info
        waits = list(si.on_wait or []) if si else []
        if any(not str(w.ant_name or "").startswith("DMA") for w in waits):
            continue
        seen[eng] = seen.get(eng, 0) + 1
        hoist.append(inst)
        tb.instructions.remove(inst)

    sem_engine = {}
    for inst in hoist:
        for u in list(inst.sync_info.on_update or []):
            sem_engine[u.id] = inst.engine

    m_insts = main.instructions
    moved = []
    for inst in list(m_insts):
        if not inst.name.startswith("I-semclr-"):
            continue
        upd = list(inst.sync_info.on_update or [])
        if upd and upd[0].id in sem_engine:
            inst.engine = sem_engine[upd[0].id]
            m_insts.remove(inst)
            moved.append(inst)

    pos = 1 if m_insts and type(m_insts[0]).__name__ == "InstCall" else 0
    for j, inst in enumerate(moved + hoist):
        m_insts.insert(pos + j, inst)


def _slim_end(nc):
    """Remove the end-block barrier: the SP engine's sem-waits (split onto
    nops) already cover every final sem value including the output-DMA
    completion, so each engine can drain and halt independently.  Exec time
    ends at the last halt (SP), saving the ~1us barrier chain.  The drains'
    gather/release wait+update are stripped so the start barrier's sems are
    untouched across re-executions."""
    f = nc.m.functions[0]
    end = [b for b in f.blocks if b.name.endswith("_end")][0]
    insts = end.instructions
    for inst in list(insts):
        tn = type(inst).__name__
        if tn == "InstEventSemaphore" and "barrier" in (inst.name or ""):
            insts.remove(inst)
        elif tn == "InstDrain":
            si = inst.sync_info
            if si is not None:
                wk = [
                    w
                    for w in (si.on_wait or [])
                    if "barrier" not in str(w.ant_name or "")
                ]
                up = [
                    u
                    for u in (si.on_update or [])
                    if "barrier" not in str(u.ant_name or "")
                ]
                inst.sync_info = mybir.SyncInfo(on_wait=wk, on_update=up)


def _tidy_main(nc):
    """Drop the dead const-ap memsets and keep the pre-barrier sem-clears
    off the busy Pool engine (round-robin PE/DVE instead)."""
    f = nc.m.functions[0]
    main = [b for b in f.blocks if b.name == "main"][0]
    m_insts = main.instructions
    k = 0
    for inst in list(m_insts):
        tn = type(inst).__name__
        if tn == "InstMemset":
            try:
                loc = str(inst.outs[0])
            except Exception:
                loc = ""
            if "const-" in loc:
                m_insts.remove(inst)
        elif inst.name.startswith("I-semclr-"):
            k += 1
            inst.engine = (
                mybir.EngineType.PE if k % 2 == 0 else mybir.EngineType.DVE
            )


def _trim_tail(nc):
    """Exec time ends when the last engine halts.  The TileContext tail is
    [drain+barrier, 20 serial sem-clears on Pool, second barrier] -- ~2.5us
    after the output DMA completes.  Re-execution of the NEFF only needs the
    sems cleared before the tile block runs, so: clear them in the MAIN
    block instead (spread across engines, before the existing all-engine
    barrier that already orders engine start), and delete the tail clears +
    second barrier."""
    f = nc.m.functions[0]
    blocks = {b.name: b for b in f.blocks}
    main = blocks["main"]
    end = [b for n, b in blocks.items() if n.endswith("_end")][0]

    insts = end.instructions
    # find the Pool drain that precedes the semclear run (after barrier-1)
    clr_idx = [i for i, x in enumerate(insts) if x.name.startswith("I-semclr-")]
    if not clr_idx:
        return
    first, last = clr_idx[0], clr_idx[-1]
    clears = insts[first : last + 1]
    # everything after the clears is barrier-2 (+ its drains): delete; also
    # delete the clears and the extra Pool drain right before them
    start_del = first
    if start_del > 0 and type(insts[start_del - 1]).__name__ == "InstDrain":
        start_del -= 1
    del insts[start_del:]

    # re-insert clears near the start of main, round-robin across engines,
    # before the all-engine barrier (the barrier orders them vs tile work)
    m_insts = main.instructions
    # insertion point: before the first InstDrain (start of the barrier)
    ins_pt = next(
        (i for i, x in enumerate(m_insts) if type(x).__name__ == "InstDrain"),
        len(m_insts),
    )
    engines = [
        mybir.EngineType.Pool,
        mybir.EngineType.DVE,
        mybir.EngineType.Activation,
        mybir.EngineType.PE,
        mybir.EngineType.SP,
    ]
    for j, c in enumerate(clears):
        c.engine = engines[j % len(engines)]
        c.sync_info = mybir.SyncInfo(
            on_wait=[], on_update=list(c.sync_info.on_update or [])[:1]
        )
        m_insts.insert(ins_pt + j, c)

# ---------------------------------------------------------------------------

B, D, NCLS, KC = 8192, 512, 90, 8
NCORES, P = 8, 128
BC = B // NCORES          # samples per core
KCH = D // P              # contraction chunks
NTILE = 512               # moving-operand columns per matmul (fp32 max)
NH = BC // NTILE          # halves per core (2)

_BUILD_CACHE = {}


def _build(nslot, post_process=True):
    M8 = 8 * nslot            # center-score rows
    M9 = 9 * nslot            # + u-candidate rows
    assert M9 <= 128, f"class slots {nslot} need {M9} > 128 partitions"
    NB = NTILE // P           # 128-sample blocks per half
    f32 = mybir.dt.float32
    bf16 = mybir.dt.bfloat16
    nc = bass.Bass("TRN2", target_bir_lowering=False, debug=False, num_devices=1)
    # xt is pre-chunked on the host: chunk (h, k) is a contiguous
    # [128, 512] block, so each chunk DMA is one linear 256 KiB read.
    xt_d = nc.dram_tensor("xt", [NH, KCH, P, NTILE], f32, kind="ExternalInput")
    # partition-major: [128, KCH, 128] so each partition row is one
    # contiguous 2 KiB read
    ct_d = nc.dram_tensor("ct", [P, KCH, P], f32, kind="ExternalInput")
    # eg: one-hot e [nslot, 1024] ++ mask-expand pattern g [nslot, 128]
    eg_d = nc.dram_tensor("eg", [nslot, BC + P], bf16, kind="ExternalInput")
    # vbsel: per-partition (scale, bias) ++ reduce stationary [128, 128]
    vbsel_d = nc.dram_tensor("vbsel", [P, 2 + P], f32, kind="ExternalInput")
    # selc: bf16 stationary for the q-part reduce (1.0 on center rows)
    selc_d = nc.dram_tensor("selc", [P, P], bf16, kind="ExternalInput")
    val_d = nc.dram_tensor("val", [2, NH, NTILE], f32, kind="ExternalOutput")

    with tile.TileContext(nc) as tc:
        with ExitStack() as ctx:
            consts = ctx.enter_context(tc.tile_pool(name="consts", bufs=1))
            pools = {
                n: ctx.enter_context(tc.tile_pool(name=n, bufs=2))
                for n in ("mkp", "vp", "tmpp", "tp", "uqp")
            }
            pwu = ctx.enter_context(tc.tile_pool(name="pwu", bufs=1, space="PSUM"))
            pst = ctx.enter_context(tc.tile_pool(name="pst", bufs=2, space="PSUM"))
            pmk = ctx.enter_context(tc.tile_pool(name="pmk", bufs=2, space="PSUM"))
            puq = ctx.enter_context(tc.tile_pool(name="puq", bufs=2, space="PSUM"))

            # warm-up zeros tile: no input dependency, so the PE can start
            # ramping its clock immediately after the gpsimd memset.
            wz = consts.tile([P, NTILE], f32)
            nc.gpsimd.memset(wz, 0.0)

            # input DMAs, spread across the 3 queue-issuing engines so issue
            # cost (~0.7us per DMA_DIRECT2D) overlaps; order within each
            # engine = order the matmuls will need the data.
            xt_sb = consts.tile([P, KCH, BC], f32)
            ct_sb = consts.tile([P, KCH, P], f32)
            eg_sb = consts.tile([nslot, BC + P], bf16)
            g_sb = eg_sb[:, BC : BC + P]
            vbsel_sb = consts.tile([P, 2 + P], f32)
            vb_sb = vbsel_sb[:, 0:2]
            sel_sb = vbsel_sb[:, 2 : 2 + P]
            selc_sb = consts.tile([P, P], bf16)
            nc.gpsimd.dma_start(out=selc_sb, in_=selc_d.ap())
            xt_ap = xt_d.ap()

            # eg (28 KiB) leads the scalar HW queue (hoisted pre-barrier
            # with ct) so the mask matmuls can open the PE's ramp window.
            nc.scalar.dma_start(out=eg_sb, in_=eg_d.ap())
            nc.scalar.dma_start(out=ct_sb, in_=ct_d.ap())

            dma_plan = [
                (nc.sync, (0, 0)), (nc.scalar, (0, 1)),
                (nc.sync, (0, 2)), (nc.scalar, (0, 3)),
                (nc.sync, (1, 0)), (nc.scalar, None),  # None = vbsel
                (nc.scalar, (1, 1)), (nc.sync, (1, 2)),
                (nc.sync, (1, 3)),
            ]
            for eng, hk in dma_plan:
                if hk is None:
                    eng.dma_start(out=vbsel_sb, in_=vbsel_d.ap())
                else:
                    h, k = hk
                    eng.dma_start(
                        out=xt_sb[:, k, h * NTILE : (h + 1) * NTILE],
                        in_=xt_ap[h, k],
                    )

            # PE warm-up: released by the wz memset only, runs while the
            # first xt chunks are in flight.  Sized to keep the PE busy
            # (HAM clock at 2.4 GHz) until chunk (0,0) lands -- any PE idle
            # gap drops the clock back and doubles the next matmuls' cost.
            wu_ps = pwu.tile([P, NTILE], f32)
            # dummy ACT op: pulls the 1.3us ACT_TABLE_LOAD off the critical
            # path (it fires before the first ACTIVATE executes)
            wact = consts.tile([1, 2], f32)
            nc.scalar.activation(
                wact, wz[0:1, 0:2], mybir.ActivationFunctionType.Identity
            )

            # mask expansion first (eg lands ~7.2us via the hoisted scalar
            # DMA): real work in the PE clock-ramp window instead of pure
            # warmup filler.  mask[r, b] = 1 iff row r is in sample b's slot.
            mask_sb = []
            for hh in range(NH):
                mk_ps = pmk.tile([P, NTILE], f32)
                nc.tensor.matmul(
                    mk_ps, g_sb,
                    eg_sb[:, hh * NTILE : (hh + 1) * NTILE],
                    start=True, stop=True,
                )
                mk = pools['mkp'].tile([P, NTILE], f32)
                nc.scalar.copy(mk, mk_ps)
                mask_sb.append(mk)
            # warmups AFTER the masks bridge the remaining wait for chunk
            # (0,0): any PE idle gap here would reset the clock ramp
            for w in range(1):
                nc.tensor.matmul(
                    wu_ps[:, 0:P], wz[:, 0:P], wz[:, 0:P], start=True,
                    stop=True, skip_group_check=True,
                )

            # main fp32 matmuls: S = table^T @ x^T, [128, 512] per half.
            # k-order 0,2,1,3 = expected DMA arrival order (k0/k2 ride the
            # sync queue, k1/k3 the scalar queue behind ct).
            KORD = [0, 2, 1, 3]
            st_ps = []
            for h in range(NH):
                sp = pst.tile([P, NTILE], f32)
                for i, k in enumerate(KORD):
                    nc.tensor.matmul(
                        sp,
                        ct_sb[:, k, :],
                        xt_sb[:, k, h * NTILE : (h + 1) * NTILE],
                        start=(i == 0),
                        stop=(i == KCH - 1),
                    )
                st_ps.append(sp)

            # epilogue per half:
            #   v    = S*scalevec + biasvec   (ACT: S on sq rows, 1 on u rows)
            #   tmp  = S * mask               (DVE)
            #   t    = tmp * v                (DVE: s^2*mask | u*mask | 0)
            #   uq   = sel^T @ t              (PE fp32: row0 = q+2u, row1 = u)
            # The (num-8, den-8) rows go straight to DRAM; the host folds
            # the +8 and the division into its existing gather/mean.
            uq_sb = consts.tile([2, NH, NTILE], f32)
            chains = []
            for h in range(NH):
                sp = st_ps[h]
                tmp = pools['tmpp'].tile([P, NTILE], f32)
                nc.vector.tensor_mul(tmp, sp, mask_sb[h])
                # tmp^2 = s^2*mask on center rows (mask is 0/1); bf16 is
                # enough for q (error ~3 abs vs ~8 budget)
                tsq = pools['tp'].tile([P, NTILE], bf16)
                nc.vector.tensor_mul(tsq, tmp, tmp)
                chains.append((tmp, tsq))
            # two-matmul reduce per half: fp32 u-part over tmp (stationary
            # sel: 2/1 on csum rows) + bf16 q-part over tmp^2 (selc: 1 on
            # center rows) accumulating into one bank.  The fp32 part only
            # needs tmp, so it starts ~1.7us earlier than the old
            # tmp->v->t chain allowed.
            for h in range(NH):
                tmp, tsq = chains[h]
                uq_ps = puq.tile([P, NTILE], f32)
                nc.tensor.matmul(
                    uq_ps, sel_sb, tmp, start=True, stop=False,
                    skip_group_check=True,
                )
                nc.tensor.matmul(
                    uq_ps, selc_sb, tsq, start=False, stop=True,
                    skip_group_check=True,
                )
                nc.scalar.copy(uq_sb[:, h, :], uq_ps[0:2])
            nc.scalar.dma_start(out=val_d.ap(), in_=uq_sb)
    if post_process:
        _rewrite_range_clears(nc)
        _trim_tail(nc)
        _tidy_main(nc)
        _hoist_first_dmas(nc)
        _split_multi_waits(nc)
        _slim_end(nc)
    return nc


def _prep_in_maps(x, centers, labels):
    import ml_dtypes

    x = np.ascontiguousarray(np.asarray(x, dtype=np.float32))
    centers = np.asarray(centers, dtype=np.float32)
    labels = np.asarray(labels).astype(np.int64)
    order = np.argsort(labels, kind="stable")
    xs = x[order]
    ls = labels[order]

    core_classes = [np.unique(ls[i * BC : (i + 1) * BC]) for i in range(NCORES)]
    nslot = max(len(c) for c in core_classes)
    M8, M9 = 8 * nslot, 9 * nslot

    # vbsel = [vb | sel]: vb = per-partition (scale, bias) so that
    # v = S on sq rows, 1.0 on u rows, 0 elsewhere; sel col0 = num-8
    # weights (1*q + 2*u), col1 = den-8 weights (1*u)
    vbsel = np.zeros((P, 2 + P), np.float32)
    vbsel[:M8, 0] = 1.0
    vbsel[M8:M9, 1] = 1.0
    vbsel[M8:M9, 2] = 2.0
    vbsel[M8:M9, 3] = 1.0
    selc = np.zeros((P, P), np.float32)
    selc[:M8, 0] = 1.0
    selc = selc.astype(ml_dtypes.bfloat16)
    # g: slot -> row-mask expansion pattern (8 center rows + 1 csum row)
    g = np.zeros((nslot, P), np.float32)
    for s in range(nslot):
        g[s, 8 * s : 8 * s + 8] = 1.0
        g[s, M8 + s] = 1.0

    in_maps = []
    for i in range(NCORES):
        sl = slice(i * BC, (i + 1) * BC)
        # chunk-contiguous layout [h, k, 128, 512] (see _build)
        xT = np.ascontiguousarray(
            xs[sl].T.reshape(KCH, P, NH, NTILE).transpose(2, 0, 1, 3)
        )
        cls = core_classes[i]
        ct = np.zeros((D, P), np.float32)
        for j, c in enumerate(cls):
            ct[:, 8 * j : 8 * j + 8] = centers[c].T
            ct[:, M8 + j] = centers[c].sum(axis=0)
        ct = np.ascontiguousarray(ct.reshape(KCH, P, P).transpose(1, 0, 2))
        slot_of = {c: j for j, c in enumerate(cls)}
        slots = np.array([slot_of[c] for c in ls[sl]])
        e = np.zeros((nslot, BC), np.float32)
        e[slots, np.arange(BC)] = 1.0
        eg = np.concatenate([e, g], axis=1).astype(ml_dtypes.bfloat16)
        in_maps.append(
            {"xt": xT, "ct": ct, "eg": eg, "vbsel": vbsel, "selc": selc}
        )
    return nslot, in_maps


def kernel(x, centers, labels, _trace=False):
    nslot, in_maps = _prep_in_maps(x, centers, labels)
    if nslot not in _BUILD_CACHE:
        _BUILD_CACHE[nslot] = _build(nslot)
    nc = _BUILD_CACHE[nslot]
    res = bass_utils.run_bass_kernel_spmd(
        nc, in_maps, core_ids=list(range(NCORES)), trace=_trace
    )
    total = 0.0
    for r in res.results:
        uq = r["val"].astype(np.float64)  # [2, NH, NTILE]: (q+2u, u), +8 off
        total += ((uq[0] + 8.0) / (uq[1] + 8.0)).sum()
    out = np.float32(total / B)
    if _trace:
        return out, res
    return out
